# revision 13
# baseline (speedup 1.0000x reference)
"""Trainium2 Bass kernel for the CoLES problem (GRU encoder + NT-Xent loss).

Strategy (8 NeuronCores, data-parallel over the K*B=1024 subsequences):
  * host: extract subsequence token ids; pregather the input-projection rows
    gx = (W_ih.T + biases)[tokens] into an fp8 stream (r/z columns x32 so the
    recurrent accumulation shares one scale); shard 128 rows/core.
  * phase 1 (per core): 64 GRU steps.  The recurrent gh matmuls run in fp8
    DoubleRow mode (contraction 256/pass: hT in fp8, W_hh.T x32 in fp8, fp32
    PSUM), 2 matmuls per gate instead of 4.  gx_r/gx_z are PSUM-injected with
    an fp8 identity matmul; the n-gate bias (x32, bf16) via a ones-row matmul.
    Tail per 256-column chunk: r/z sigmoids on ACT with scale=1/32 straight
    from PSUM; p = (ps_n/32)*r fused on DVE (scalar_tensor_tensor);
    t_n = p + gx_n on GpSimd; n = tanh on ACT; h' = n + z*(h-n) on DVE
    (3 tensor ops, bf16 2x mode); h' transposed on PE (bf16 PSUM) and copied
    to an fp8 hT tile (2 copies DVE / 2 ACT, cast in the copy).
    Mean-pool: pair/quad partial sums of fp8 hT on GpSimd (bf16 out),
    projected into an fp32 PSUM accumulator once per 4 steps (bf16 matmuls,
    priority-demoted).
  * host glue: add proj_b, L2-normalize, build fp8 phase-2 operands (x16).
  * phase 2 (per core): 128x1024 block of the similarity matrix via one fp8
    DoubleRow matmul per 512-column half; es=exp(sim/(256*tau)) with fused
    row-sum; fused row-sum of es*pos (pos mask in fp8); exp table pre-warmed
    under the input DMAs; host es_ii correction uses the same fp8 zn.
  * host: loss = mean(-log(num/(den+1e-10)+1e-10)) with has_pos handling.
"""
import os
import sys

sys.path.insert(0, "/opt/trn_rl_repo")

import numpy as np
import ml_dtypes

import concourse.bass as bass
import concourse.tile as tile
from concourse import bacc, mybir
from concourse.masks import make_identity
from concourse.bass_utils import run_bass_kernel_spmd

BF = mybir.dt.bfloat16
F32 = mybir.dt.float32
FP8 = mybir.dt.float8e4

B, S, V, H, E, L, K = 512, 512, 1024, 512, 256, 64, 2
TAU = 0.1
NCORES = 8
N = K * B
NLOC = N // NCORES  # 128
SW = 32.0           # fp8 weight/gx scale; descaled in the activations
ZS = 16.0           # phase-2 zn scale (sim accumulates ZS*ZS = 256)

SIG = mybir.ActivationFunctionType.Sigmoid
TANH = mybir.ActivationFunctionType.Tanh
EXP = mybir.ActivationFunctionType.Exp
DR = mybir.MatmulPerfMode.DoubleRow

np_bf16 = ml_dtypes.bfloat16
np_fp8 = ml_dtypes.float8_e4m3  # TRN fp8e4 variant (max 240)


def _build_phase1(steps=L, gx_bufs=6, demote_pool=100000):
    nc = bacc.Bacc("TRN2", target_bir_lowering=False, debug=False)
    gx8 = nc.dram_tensor("gx8", [128, steps, 3 * H], FP8, kind="ExternalInput").ap()
    # DR layout: whh8[pass][p, i, g*512+n] = SW * W_hh.T[pass*256 + i*128 + p, ...]
    whh8 = nc.dram_tensor("whh8", [128, 2, 2, 3 * H], FP8, kind="ExternalInput").ap()
    bhhn = nc.dram_tensor("bhhn", [1, H], BF, kind="ExternalInput").ap()
    projwt = nc.dram_tensor("projwt", [H, E], BF, kind="ExternalInput").ap()
    zraw = nc.dram_tensor("zraw", [NLOC, E], F32, kind="ExternalOutput").ap()

    with tile.TileContext(nc) as tc:
        with (
            tc.tile_pool(name="singles", bufs=1) as singles,
            tc.tile_pool(name="gx", bufs=gx_bufs) as gxp,
            tc.tile_pool(name="state", bufs=2) as statep,
            tc.tile_pool(name="gates", bufs=2) as gatep,
            tc.tile_pool(name="psg", bufs=2, space="PSUM") as psg,
            tc.tile_pool(name="psT", bufs=1, space="PSUM") as psT,
            tc.tile_pool(name="psE", bufs=1, space="PSUM") as psE,
        ):
            whh8_sb = []
            for p in range(2):
                t = singles.tile([128, 2, 3 * H], FP8, tag=f"whh8_{p}")
                nc.sync.dma_start(t[:], whh8[:, p, :, :])
                whh8_sb.append(t)
            projwt_sb = []
            for c in range(4):
                t = singles.tile([128, E], BF, tag=f"projwt{c}")
                nc.sync.dma_start(t[:], projwt[c * 128:(c + 1) * 128, :])
                projwt_sb.append(t)
            bhhn_sb = singles.tile([1, H], BF, tag="bhhn")
            nc.sync.dma_start(bhhn_sb[:], bhhn[:])
            ones1 = singles.tile([1, 128], BF, tag="ones1")
            nc.vector.memset(ones1[:], 1.0)
            ident = singles.tile([128, 128], BF, tag="ident")
            make_identity(nc, ident[:])
            ident8 = singles.tile([128, 128], FP8, tag="ident8")
            make_identity(nc, ident8[:])
            hzero = singles.tile([128, H], BF, tag="hzero")
            nc.vector.memset(hzero[:], 0.0)

            emb_ps = psE.tile([128, E], F32, tag="emb")
            # psT is one 2KB bank holding both parities: step l writes slots
            # (l%2)*4+c so step l+1's transposes don't WAR-stall on l's copies
            psT_t = psT.tile([128, 8, 128], BF, tag="ht")

            # Keep-alive dummies: the PE DVFS governor drops to the 1.2GHz
            # p-state whenever the engine idles for a few hundred ns, and
            # takes ~2-3us of continuous execution to ramp back.  With DR
            # halving the real matmul stream the PE never ramps and every
            # matmul runs at half clock.  Zero-stationary matmuls that
            # accumulate 0 into the (open) emb bank are correctness-free
            # filler that pins the p-state through the chain stalls.
            def dummy(n=1, cols=E):
                for _ in range(n):
                    nc.tensor.matmul(emb_ps[:, 0:cols], hzero[:, 0:128],
                                     projwt_sb[0][:, 0:cols],
                                     start=False, stop=False,
                                     skip_group_check=True)

            h_prev = hzero
            hT_prev = None
            nproj = 0
            gx_tile = None
            A = mybir.AluOpType
            for l in range(steps):
                if l % 2 == 0:
                    gx_tile = gxp.tile([128, 2, 3 * H], FP8, tag="gx")
                    nc.sync.dma_start(gx_tile[:], gx8[:, l:l + 2, :])
                gl = l % 2
                par = 4 * (l % 2)

                # injections first (start each gate's accumulation group),
                # then the fp8 DoubleRow gh matmuls: 2 passes of 256
                # contraction (hT chunk pairs), pass-major so the hT halves
                # (available in copy order: DVE c01, ACT c23) are consumed
                # in arrival order.  DR issues at the same 216ns as bf16 but
                # contracts 2x per pass -> 6 matmuls instead of 12.
                ps_r = psg.tile([128, H], F32, tag="psr")
                ps_z = psg.tile([128, H], F32, tag="psz")
                ps_n = psg.tile([128, H], F32, tag="psn")
                nc.tensor.matmul(ps_r[:], ident8[:], gx_tile[:, gl, 0:H],
                                 start=True, stop=(l == 0))
                nc.tensor.matmul(ps_n[:], ones1[:], bhhn_sb[:], start=True,
                                 stop=(l == 0))
                nc.tensor.matmul(ps_z[:], ident8[:], gx_tile[:, gl, H:2 * H],
                                 start=True, stop=(l == 0))
                if l == 0:
                    # open the emb accumulation group (pool matmuls and
                    # keep-alive dummies all join with start=False)
                    nc.tensor.matmul(emb_ps[:], hzero[:, 0:128],
                                     projwt_sb[0][:], start=True, stop=False,
                                     skip_group_check=True)
                if l > 0:
                    dummy(2)
                    for p in range(2):
                        hT_p = hT_prev[:, 2 * p:2 * p + 2, :]
                        for ps, g in ((ps_r, 0), (ps_n, 2), (ps_z, 1)):
                            nc.tensor.matmul(
                                ps[:], hT_p,
                                whh8_sb[p][:, :, g * H:(g + 1) * H],
                                start=False, stop=(p == 1), perf_mode=DR)
                        if p == 0 and l < steps - 1:
                            dummy(3)
                    if l < steps - 1:
                        dummy(3)

                r = gatep.tile([128, H], BF, tag="r")
                z = gatep.tile([128, H], BF, tag="z")
                h_new = statep.tile([128, H], BF, tag="h")
                hT_new = statep.tile([128, 4, 128], FP8, tag="hT")
                c0, c1 = slice(0, 256), slice(256, 512)

                # ACT: r sigmoids (chunked, chain-first), z unchunked between
                nc.scalar.activation(r[:, c0], ps_r[:, c0], SIG, scale=1.0 / SW)
                nc.scalar.activation(r[:, c1], ps_r[:, c1], SIG, scale=1.0 / SW)
                nc.scalar.activation(z[:], ps_z[:], SIG, scale=1.0 / SW)

                # The whole elementwise chain lives on DVE (fastest engine,
                # in-order, no cross-engine queue blocking): p/tn for both
                # chunks first (in readiness order), then the c0 tail, then
                # the c1 tail.  GpSimd only does the off-chain CAST of hT01;
                # ACT keeps the sigmoids/tanhs and the late COPY of hT23.
                p0 = gatep.tile([128, 256], BF, tag="p0")
                nc.vector.scalar_tensor_tensor(p0[:], ps_n[:, c0], 1.0 / SW,
                                               r[:, c0], op0=A.mult, op1=A.mult)
                tn0 = gatep.tile([128, 256], BF, tag="tn0")
                nc.vector.tensor_add(tn0[:], p0[:], gx_tile[:, gl, 2 * H:2 * H + 256])
                p1 = gatep.tile([128, 256], BF, tag="p1")
                nc.vector.scalar_tensor_tensor(p1[:], ps_n[:, c1], 1.0 / SW,
                                               r[:, c1], op0=A.mult, op1=A.mult)
                tn1 = gatep.tile([128, 256], BF, tag="tn1")
                nc.vector.tensor_add(tn1[:], p1[:],
                                     gx_tile[:, gl, 2 * H + 256:3 * H])
                ng0 = gatep.tile([128, 256], BF, tag="ng0")
                nc.scalar.activation(ng0[:], tn0[:], TANH)
                ng1 = gatep.tile([128, 256], BF, tag="ng1")
                nc.scalar.activation(ng1[:], tn1[:], TANH)

                # h' = n + z*(h-n): both chunks' {d,m,h} on DVE back-to-back
                d0 = gatep.tile([128, 256], BF, tag="d0")
                nc.vector.tensor_sub(d0[:], h_prev[:, c0], ng0[:])
                m0 = gatep.tile([128, 256], BF, tag="m0")
                nc.vector.tensor_mul(m0[:], z[:, c0], d0[:])
                nc.vector.tensor_add(h_new[:, c0], ng0[:], m0[:])
                nc.tensor.transpose(psT_t[:, par + 0, :], h_new[:, 0:128], ident[:])
                nc.tensor.transpose(psT_t[:, par + 1, :], h_new[:, 128:256], ident[:])
                nc.vector.tensor_copy(hT_new[:, 0:2, :], psT_t[:, par:par + 2, :])

                d1 = gatep.tile([128, 256], BF, tag="d1")
                nc.vector.tensor_sub(d1[:], h_prev[:, c1], ng1[:])
                m1 = gatep.tile([128, 256], BF, tag="m1")
                nc.vector.tensor_mul(m1[:], z[:, c1], d1[:])
                nc.vector.tensor_add(h_new[:, c1], ng1[:], m1[:])
                nc.tensor.transpose(psT_t[:, par + 2, :], h_new[:, 256:384], ident[:])
                nc.tensor.transpose(psT_t[:, par + 3, :], h_new[:, 384:512], ident[:])
                nc.scalar.copy(hT_new[:, 2:4, :], psT_t[:, par + 2:par + 4, :])

                # pooling: project each step's fp8 hT chunks straight into
                # the fp32 PSUM accumulator (priority-demoted so they fill
                # PE gaps; they double as p-state keep-alive work)
                _p0 = tc.cur_priority
                tc.cur_priority = _p0 + demote_pool
                for c in range(4):
                    nc.tensor.matmul(emb_ps[:], hT_new[:, c, :],
                                     projwt_sb[c][:],
                                     start=False,
                                     stop=(l == steps - 1 and c == 3),
                                     skip_group_check=True)
                    nproj += 1
                tc.cur_priority = _p0

                h_prev = h_new
                hT_prev = hT_new

            zsb = singles.tile([128, E], F32, tag="zout")
            nc.scalar.copy(zsb[:], emb_ps[:])
            nc.sync.dma_start(zraw[:], zsb[:])

    nc.compile()
    return nc


def _build_phase2():
    nc = bacc.Bacc("TRN2", target_bir_lowering=False, debug=False)
    znt = nc.dram_tensor("znt", [128, 2, N], FP8, kind="ExternalInput").ap()
    zntl = nc.dram_tensor("zntl", [128, 2, 128], FP8, kind="ExternalInput").ap()
    posm = nc.dram_tensor("posm", [128, N], FP8, kind="ExternalInput").ap()
    nd = nc.dram_tensor("nd", [128, 2], F32, kind="ExternalOutput").ap()

    with tile.TileContext(nc) as tc:
        with (
            tc.tile_pool(name="sb", bufs=1) as sb,
            tc.tile_pool(name="ps", bufs=2, space="PSUM") as ps,
        ):
            junk = sb.tile([128, 512], F32, tag="junk")
            # warm the exp activation table while the input DMAs run
            warm = sb.tile([128, 8], F32, tag="warm")
            nc.vector.memset(warm[:], 0.0)
            nc.scalar.activation(warm[:], warm[:], EXP)
            znt_sb = sb.tile([128, 2, N], FP8, tag="znt")
            nc.sync.dma_start(znt_sb[:], znt[:])
            zntl_sb = sb.tile([128, 2, 128], FP8, tag="zntl")
            nc.sync.dma_start(zntl_sb[:], zntl[:])
            posm_sb = sb.tile([128, N], FP8, tag="posm")
            nc.sync.dma_start(posm_sb[:], posm[:])

            s_parts, n_parts = [], []
            for half in range(2):
                pst = ps.tile([128, 512], F32, tag="sim")
                nc.tensor.matmul(pst[:], zntl_sb[:],
                                 znt_sb[:, :, half * 512:(half + 1) * 512],
                                 start=True, stop=True, perf_mode=DR)
                es = sb.tile([128, 512], F32, tag=f"es{half}")
                s_p = sb.tile([128, 1], F32, tag=f"sp{half}")
                nc.scalar.activation(es[:], pst[:], EXP,
                                     scale=1.0 / (ZS * ZS * TAU),
                                     accum_out=s_p[:])
                n_p = sb.tile([128, 1], F32, tag=f"np{half}")
                nc.vector.scalar_tensor_tensor(
                    junk[:], es[:], 1.0, posm_sb[:, half * 512:(half + 1) * 512],
                    op0=mybir.AluOpType.mult, op1=mybir.AluOpType.mult,
                    accum_out=n_p[:])
                s_parts.append(s_p)
                n_parts.append(n_p)

            out_sb = sb.tile([128, 2], F32, tag="out")
            nc.vector.tensor_add(out_sb[:, 0:1], n_parts[0][:], n_parts[1][:])
            nc.vector.tensor_add(out_sb[:, 1:2], s_parts[0][:], s_parts[1][:])
            nc.sync.dma_start(nd[:], out_sb[:])

    nc.compile()
    return nc


_CACHE = {}

# Filled by kernel() on every call: [("phase1", BassKernelResults), ...].
# exec_time_ns is populated when the KERNEL_PROFILE env var is set.
LAST_RESULTS = []


def _get_programs():
    if "nc1" not in _CACHE:
        _CACHE["nc1"] = _build_phase1()
        _CACHE["nc2"] = _build_phase2()
    return _CACHE["nc1"], _CACHE["nc2"]


def _run(nc, in_maps, name):
    kw = {}
    if os.environ.get("KERNEL_PROFILE"):
        kw = dict(trace=True)
        d = os.environ.get("KERNEL_PROFILE_DIR")
        if d:
            kw["tmpdir"] = os.path.join(d, name)
            os.makedirs(kw["tmpdir"], exist_ok=True)
    res = run_bass_kernel_spmd(nc, in_maps, core_ids=list(range(NCORES)), **kw)
    LAST_RESULTS.append((name, res))
    return res


def _prep1(sequence, starts, W_ih, W_hh, b_ih, b_hh, proj_W):
    """Host prep for phase 1: pregathered fp8 gx stream + fp8 weights."""
    idx = starts[:, :, None].astype(np.int64) + np.arange(L)[None, None, :]
    sub = sequence[np.arange(B)[None, :, None], idx].reshape(N, L)

    bcomb = np.concatenate([
        b_ih[:H] + b_hh[:H], b_ih[H:2 * H] + b_hh[H:2 * H], b_ih[2 * H:]
    ]).astype(np.float32)
    tab = W_ih.T.astype(np.float32) + bcomb[None, :]
    tab[:, :2 * H] *= SW
    tab8 = tab.astype(np_fp8)
    gx_full = tab8[sub]                       # [N, L, 3H] fp8

    # DR stationary-pair layout: [128, pass, i, 3H]
    w8 = (W_hh.T.astype(np.float32) * SW).astype(np_fp8)
    whh8 = w8.reshape(2, 2, 128, 3 * H).transpose(2, 0, 1, 3)

    shared = dict(
        whh8=np.ascontiguousarray(whh8),
        bhhn=np.ascontiguousarray(
            (b_hh[2 * H:] * SW).reshape(1, H).astype(np_bf16)),
        projwt=np.ascontiguousarray(
            (proj_W.T.astype(np.float32) / L).astype(np_bf16)),
    )
    in_maps = []
    for c in range(NCORES):
        m = dict(shared)
        m["gx8"] = np.ascontiguousarray(gx_full[c * NLOC:(c + 1) * NLOC])
        in_maps.append(m)
    return in_maps


def _prep2(z, labels, proj_b):
    """Host glue: add bias, normalize, build fp8 phase-2 operands."""
    lab = np.tile(labels, K)
    z = z + proj_b[None, :].astype(np.float32)
    norm = np.maximum(np.sqrt((z ** 2).sum(1, keepdims=True)), 1e-12)
    zn = (z / norm).astype(np.float32)
    zn8 = (zn * ZS).astype(np_fp8)
    znt_r = np.ascontiguousarray(
        zn8.T.reshape(2, 128, N).transpose(1, 0, 2))   # [128, 2, N]
    pos = (lab[None, :] == lab[:, None]) & ~np.eye(N, dtype=bool)
    posf = pos.astype(np_fp8)

    in_maps = []
    for c in range(NCORES):
        in_maps.append(dict(
            znt=znt_r,
            zntl=np.ascontiguousarray(znt_r[:, :, c * NLOC:(c + 1) * NLOC]),
            posm=posf[c * NLOC:(c + 1) * NLOC, :],
        ))
    return in_maps, zn8, pos


def _final(nd, zn8, pos):
    """Host: assemble the NT-Xent loss from per-row num / row-sum."""
    num = nd[:, 0].astype(np.float64)
    ssum = nd[:, 1].astype(np.float64)
    q = zn8.astype(np.float64)
    es_ii = np.exp((q * q).sum(1) / (ZS * ZS * TAU))
    den = ssum - es_ii
    has_pos = pos.any(1)
    li = -np.log(num / (den + 1e-10) + 1e-10)
    loss = np.where(has_pos, li, 0.0).sum() / max(int(has_pos.sum()), 1)
    return np.float32(loss)


def kernel(sequence, labels, starts, W_ih, W_hh, b_ih, b_hh, proj_W, proj_b):
    sequence = np.asarray(sequence)
    labels = np.asarray(labels)
    starts = np.asarray(starts)
    W_ih = np.asarray(W_ih, np.float32)
    W_hh = np.asarray(W_hh, np.float32)
    b_ih = np.asarray(b_ih, np.float32)
    b_hh = np.asarray(b_hh, np.float32)
    proj_W = np.asarray(proj_W, np.float32)
    proj_b = np.asarray(proj_b, np.float32)

    nc1, nc2 = _get_programs()
    in_maps1 = _prep1(sequence, starts, W_ih, W_hh, b_ih, b_hh, proj_W)

    LAST_RESULTS.clear()
    res1 = _run(nc1, in_maps1, "phase1")
    z = np.concatenate([res1.results[c]["zraw"] for c in range(NCORES)], 0)

    in_maps2, zn8, pos = _prep2(z, labels, proj_b)
    res2 = _run(nc2, in_maps2, "phase2")
    nd = np.concatenate([res2.results[c]["nd"] for c in range(NCORES)], 0)
    return _final(nd, zn8, pos)

